# revision 1
# baseline (speedup 1.0000x reference)
"""Boundary rendering module for Trainium2 (8 NeuronCores).

Computes, for x of shape (2, 4, 64, 256, 256) f32:
    mn/mx  = per-channel global min/max
    binary = ((x - mn) / (mx - mn)) > 0.5     [== (x - mn) > 0.5*(mx - mn)]
    dilated = 3x3x3 binary dilation of binary (SAME padding)
    out    = dilated - binary

Sharding: H (=256) split into 8 chunks of 32 rows, one per NeuronCore.
Each core receives its 32 rows plus one halo row on each side (global
edges padded with -1e30 so the halo mask is 0).  On-core layout puts
(B, D) = 128 on the SBUF partition axis; (C, H, W) live on the free axis.

Per-channel min/max: per-partition partials on DVE, transposed across
partitions with a tiny PE matmul against an identity, reduced on DVE,
then an 8-core AllReduce(max) collective over the pair (mx, -mn), and
broadcast back to 128 partitions with a rank-1 PE matmul.

The 3x3x3 dilation is computed as a count:
    count[p, h, w] = sum_{dw in {-1,0,1}} sum_q bandA[p,q] * mH[q, h, w+dw]
where mH is the H-dilated binary mask (2 vector max ops) and bandA is the
(b,d)-banded 0/1 matrix (D-axis window).  The W shifts are plain +-1 column
offsets into a zero-padded mask buffer, accumulated in PSUM by TensorE.
A final accumulating matmul adds -16 * binary, so
    psum >= 1  iff  count >= 1 and binary == 0
which a single saturated sigmoid activation turns into exact {0.0, 1.0}.
"""

import os
import sys

import numpy as np

for _p in ("/opt/trn_rl_repo", "/root/.axon_site/_ro/trn_rl_repo"):
    if os.path.isdir(_p) and _p not in sys.path:
        sys.path.insert(0, _p)

import ml_dtypes

B, C, D, H, W = 2, 4, 64, 256, 256
NCORES = 8
HS = H // NCORES  # 32 own rows per core
HA = HS + 2  # rows incl halo
HPAD = np.float32(-1e30)  # halo pad at global H edges -> mask 0

MHW = 258  # mH row width: 256 data cols + 2 zero pad cols
MHLEN = 33 * MHW + 2  # 33 rows (1 pad + 32 data) + slack for dw=+1 AP views

_CACHE = {}


def _consts():
    bd = np.arange(128)
    b = bd // D
    d = bd % D
    A = (b[:, None] == b[None, :]) & (np.abs(d[:, None] - d[None, :]) <= 1)
    A = A.astype(ml_dtypes.bfloat16)
    negI = (-16.0 * np.eye(128)).astype(ml_dtypes.bfloat16)
    I128 = np.eye(128, dtype=np.float32)
    return A, negI, I128


def _build(reps: int = 1, phase: str = "B", parts: str = "all"):
    import concourse.bass as bass
    import concourse.bacc as bacc
    import concourse.mybir as mybir
    import concourse.tile as tile
    from contextlib import ExitStack

    f32 = mybir.dt.float32
    bf16 = mybir.dt.bfloat16
    Alu = mybir.AluOpType

    nc = bacc.Bacc(
        "TRN2",
        target_bir_lowering=False,
        debug=False,
        num_devices=NCORES,
    )

    xs = nc.dram_tensor("xs", [B, C, D, HA, W], f32, kind="ExternalInput")
    if phase == "A":
        pmm = nc.dram_tensor("pmm", [8, 1], f32, kind="ExternalOutput")
        out = pm64 = None
    else:
        pm64 = nc.dram_tensor("pm64", [NCORES, 8], f32, kind="ExternalInput")
        out = nc.dram_tensor("out", [B, C, D, HS, W], f32, kind="ExternalOutput")
        pmm = None
    A_np, negI_np, I_np = _consts()
    bandA_d = nc.inline_tensor(A_np, name="bandA")
    negI_d = nc.inline_tensor(negI_np, name="negI")
    ident_d = nc.inline_tensor(I_np, name="ident")

    # partition axis = (b, d) = 128; DRAM-side APs keep b and d as separate
    # leading dims (DMA pairs elements in iteration order, b-major then d,
    # matching partition index p = b*64 + d).
    xsa = xs.ap()
    outa = out.ap() if out is not None else None

    with ExitStack() as ctx:
        tc = ctx.enter_context(tile.TileContext(nc))
        pers = ctx.enter_context(tc.tile_pool(name="pers", bufs=1))
        psump = ctx.enter_context(tc.tile_pool(name="psum", bufs=2, space="PSUM"))

        x_all = pers.tile([128, C, HA, W], f32)  # 136 KiB / partition
        binm = pers.tile([128, HA, W], bf16)  # 17 KiB  {0,1}
        mH = pers.tile([128, MHLEN], bf16)  # ~16.7 KiB  H-dilated mask
        stag = pers.tile([128, 4096], f32)  # out staging (16 rows)
        pmax = pers.tile([128, 16], f32)
        pmin = pers.tile([128, 16], f32)
        red8 = pers.tile([128, 8], f32)  # [mx(4) | -mn(4)] local
        s8 = pers.tile([128, 1], f32)  # per-partition reduced (parts 0..7)
        s1v = pers.tile([128, 72], f32)  # gathered partials + reduced vals
        gv8 = pers.tile([128, 8], f32)  # broadcast [mx | -mn] on all parts
        mnv = pers.tile([128, 4], f32)  # mn per channel
        h4 = pers.tile([128, 4], f32)  # 0.5*(mx-mn) per channel
        At = pers.tile([128, 128], bf16)
        Nt = pers.tile([128, 128], bf16)
        It = pers.tile([128, 128], f32)
        ones1 = pers.tile([128, 128], f32)  # row 0 used as all-ones lhsT
        sel_bias = pers.tile([128, 1], f32)

        nc.vector.memset(sel_bias[:, :], -100.0)
        nc.vector.memset(ones1[:, :], 1.0)
        nc.gpsimd.dma_start(out=At[:, :], in_=bandA_d.ap())
        nc.gpsimd.dma_start(out=Nt[:, :], in_=negI_d.ap())
        nc.gpsimd.dma_start(out=It[:, :], in_=ident_d.ap())
        nc.vector.memset(mH[:, :], 0.0)  # zero pads once; data rows rewritten
        if parts != "all":
            # stage-isolated benchmark builds: pre-write every cross-stage
            # buffer once so skipped producers don't leave unwritten reads
            nc.vector.memset(x_all[:, :, :, :], 0.0)
            nc.vector.memset(binm[:, :, :], 0.0)
            nc.vector.memset(stag[:, :], 0.0)
            nc.vector.memset(s8[:, :], 0.0)
            nc.vector.memset(s1v[:, :], 0.0)
            nc.vector.memset(red8[:, :], 0.0)
            nc.vector.memset(pmax[:, :], 0.0)
            nc.vector.memset(pmin[:, :], 0.0)

        for _rep in range(reps):
            if phase == "A":
                _pass_a(
                    nc, mybir, Alu, psump, xsa, pmm,
                    x_all, pmax, pmin, red8, s8, It, parts,
                )
            else:
                _pass_b(
                    nc, mybir, Alu, psump, xsa, outa, pm64,
                    x_all, binm, mH, stag, s1v, gv8,
                    mnv, h4, At, Nt, ones1, sel_bias, parts,
                )

    nc.compile()
    return nc


def _load_x(nc, xsa, x_all, parts):
    engines = [nc.sync, nc.scalar]
    for i in range(8):
        c, half = i // 2, i % 2
        eng = engines[i % len(engines)]
        eng.dma_start(
            out=x_all[:, c, 17 * half : 17 * half + 17, :],
            in_=xsa[:, c, :, 17 * half : 17 * half + 17, :],
        )


def _pass_a(
    nc, mybir, Alu, psump, xsa, pmm,
    x_all, pmax, pmin, red8, s8, It, parts="all",
):
    """Load the shard and reduce it to [mx(4) | -mn(4)] -> DRAM pmm[8,1]."""
    f32 = mybir.dt.float32
    on = lambda p: parts == "all" or p in parts
    if on("dma"):
        _load_x(nc, xsa, x_all, parts)
    else:
        # lite loads: defeat cross-rep CSE/DCE while costing ~nothing
        for c in range(C):
            nc.sync.dma_start(out=x_all[:, c, 0, :], in_=xsa[:, c, :, 0, :])
    if not on("dve"):
        nc.sync.dma_start(out=pmm.ap(), in_=s8[0:8, 0:1])
        return
    for c in range(C):
        for k in range(4):
            chunk = x_all[:, c, 1 + 8 * k : 9 + 8 * k, :]
            nc.vector.tensor_reduce(
                out=pmax[:, 4 * c + k : 4 * c + k + 1],
                in_=chunk,
                axis=mybir.AxisListType.XY,
                op=Alu.max,
            )
            nc.vector.tensor_reduce(
                out=pmin[:, 4 * c + k : 4 * c + k + 1],
                in_=chunk,
                axis=mybir.AxisListType.XY,
                op=Alu.min,
            )
    for c in range(C):
        nc.vector.tensor_reduce(
            out=red8[:, c : c + 1],
            in_=pmax[:, 4 * c : 4 * c + 4],
            axis=mybir.AxisListType.X,
            op=Alu.max,
        )
        nc.vector.tensor_reduce(
            out=red8[:, 4 + c : 5 + c],
            in_=pmin[:, 4 * c : 4 * c + 4],
            axis=mybir.AxisListType.X,
            op=Alu.min,
        )
    # negate the mins so a single max combines both downstream
    nc.vector.tensor_scalar_mul(red8[:, 4:8], red8[:, 4:8], -1.0)
    # cross-partition max: transpose red8 [128p, 8] -> psum [8p, 128] with a
    # PE matmul against the identity, then free-axis reduce on DVE.
    pst = psump.tile([128, 2048], f32, tag="ps")
    nc.tensor.matmul(pst[0:8, 0:128], red8[:, :], It[:, :], start=True, stop=True)
    nc.vector.tensor_reduce(
        out=s8[0:8, 0:1],
        in_=pst[0:8, 0:128],
        axis=mybir.AxisListType.X,
        op=Alu.max,
    )
    nc.sync.dma_start(out=pmm.ap(), in_=s8[0:8, 0:1])


def _pass_b(
    nc, mybir, Alu, psump, xsa, outa, pm64,
    x_all, binm, mH, stag, s1v, gv8,
    mnv, h4, At, Nt, ones1, sel_bias, parts="all",
):
    """Main pipeline: thresholds from pm64, mask, dilate, boundary."""
    f32 = mybir.dt.float32
    on = lambda p: parts == "all" or p in parts
    if on("dma"):
        for c in range(C):
            nc.sync.dma_start(out=x_all[:, c, :, :], in_=xsa[:, c, :, :, :])
    else:
        for c in range(C):
            nc.sync.dma_start(out=x_all[:, c, 0, :], in_=xsa[:, c, :, 0, :])

    # reduce the gathered per-core partials [8 cores, 8] over cores on
    # partition 0, then broadcast to all partitions with a rank-1 matmul.
    nc.sync.dma_start(out=s1v[0:1, 0:64], in_=pm64.ap().rearrange("k j -> (k j)")[None, :])
    nc.vector.tensor_reduce(
        out=s1v[0:1, 64:72],
        in_=s1v[0:1, 0:64].rearrange("p (k j) -> p j k", k=NCORES),
        axis=mybir.AxisListType.X,
        op=Alu.max,
    )
    psb = psump.tile([128, 2048], f32, tag="ps")
    nc.tensor.matmul(psb[:, 0:8], ones1[0:1, :], s1v[0:1, 64:72], start=True, stop=True)
    nc.vector.tensor_copy(gv8[:, :], psb[:, 0:8])
    nc.vector.tensor_scalar_mul(mnv[:, :], gv8[:, 4:8], -1.0)
    nc.vector.tensor_add(h4[:, :], gv8[:, 0:4], gv8[:, 4:8])
    nc.vector.tensor_scalar_mul(h4[:, :], h4[:, :], 0.5)

    # ---- mask, dilate, boundary ----
    mHd = mH[:, MHW : MHW + 32 * MHW].rearrange("p (r z) -> p r z", z=MHW)[
        :, :, 0:W
    ]
    for c in range(C):
        if on("dve"):
            nc.vector.tensor_scalar(
            out=binm[:, :, :],
            in0=x_all[:, c, :, :],
            scalar1=mnv[:, c : c + 1],
            scalar2=h4[:, c : c + 1],
            op0=Alu.subtract,
            op1=Alu.is_gt,
            )
            nc.vector.tensor_tensor(
                out=mHd,
                in0=binm[:, 0:HS, :],
                in1=binm[:, 2 : HS + 2, :],
                op=Alu.max,
            )
            nc.vector.tensor_tensor(
                out=mHd,
                in0=mHd,
                in1=binm[:, 1 : HS + 1, :],
                op=Alu.max,
            )
            if not on("pe"):
                # tiny live consumer of mH so DCE keeps the masks
                nc.vector.tensor_reduce(
                    out=stag[:, c : c + 1],
                    in_=mH[:, 0:128],
                    axis=mybir.AxisListType.X,
                    op=Alu.max,
                )
        for t in range(2):  # 16 own rows per staging buffer
            ps = psump.tile([128, 2048], f32, tag="ps")
            ps2 = psump.tile([128, 2048], f32, tag="ps")
            for half, pst_ in ((0, ps), (1, ps2)):
                if on("pe"):
                    for s in range(4):  # one PSUM bank = 2 rows = 512
                        R = 16 * t + 8 * half + 2 * s
                        pslice = pst_[:, 512 * s : 512 * s + 512]
                        for j, dw in enumerate((-1, 0, 1)):
                            off = (R + 1) * MHW + dw
                            rhs = mH[:, off : off + 2 * MHW].rearrange(
                                "p (r z) -> p r z", z=MHW
                            )[:, :, 0:W]
                            nc.tensor.matmul(
                                pslice,
                                At[:, :],
                                rhs,
                                start=(j == 0),
                                stop=False,
                            )
                        nc.tensor.matmul(
                            pslice,
                            Nt[:, :],
                            binm[:, 1 + R : 3 + R, :],
                            start=False,
                            stop=True,
                        )
                if on("pe") and not on("act"):
                    nc.vector.tensor_copy(
                        out=stag[:, 4 + 2 * half : 5 + 2 * half], in_=pst_[:, 0:1]
                    )
                if on("act"):
                    nc.scalar.activation(
                        out=stag[:, 2048 * half : 2048 * half + 2048],
                        in_=pst_[:, :],
                        func=mybir.ActivationFunctionType.Sigmoid,
                        bias=sel_bias[:, :],
                        scale=200.0,
                    )
            if on("store"):
                eng = nc.sync if (2 * c + t) % 2 == 0 else nc.scalar
                eng.dma_start(
                    out=outa[:, c, :, 16 * t : 16 * t + 16, :],
                    in_=stag.rearrange("p (r w) -> p r w", w=W),
                )


def _get_nc(phase="B"):
    key = "nc" + phase
    if key not in _CACHE:
        _CACHE[key] = _build(phase=phase)
    return _CACHE[key]


def _make_in_maps(x: np.ndarray):
    in_maps = []
    for k in range(NCORES):
        xs = np.empty((B, C, D, HA, W), np.float32)
        lo = k * HS
        xs[:, :, :, 1 : HS + 1, :] = x[:, :, :, lo : lo + HS, :]
        if k > 0:
            xs[:, :, :, 0, :] = x[:, :, :, lo - 1, :]
        else:
            xs[:, :, :, 0, :] = HPAD
        if k < NCORES - 1:
            xs[:, :, :, HS + 1, :] = x[:, :, :, lo + HS, :]
        else:
            xs[:, :, :, HS + 1, :] = HPAD
        in_maps.append({"xs": xs})
    return in_maps


def kernel(x: np.ndarray) -> np.ndarray:
    from concourse.bass_utils import run_bass_kernel_spmd

    x = np.ascontiguousarray(np.asarray(x), dtype=np.float32)
    assert x.shape == (B, C, D, H, W)

    in_maps = _make_in_maps(x)
    cores = list(range(NCORES))

    # launch A: per-core min/max partials
    res_a = run_bass_kernel_spmd(_get_nc("A"), in_maps, core_ids=cores)
    pm64 = np.concatenate(
        [res_a.results[k]["pmm"].reshape(1, 8) for k in range(NCORES)], axis=0
    )

    # launch B: full pipeline with the gathered partials
    in_maps_b = [{"xs": m["xs"], "pm64": pm64} for m in in_maps]
    res = run_bass_kernel_spmd(_get_nc("B"), in_maps_b, core_ids=cores)
    pieces = [res.results[k]["out"] for k in range(NCORES)]
    return np.concatenate(pieces, axis=3)


if __name__ == "__main__":
    x = np.random.randn(B, C, D, H, W).astype(np.float32)
    y = kernel(x)
    print(y.shape, y.dtype, y.sum())



# revision 17
# speedup vs baseline: 1.2351x; 1.2351x over previous
"""Boundary rendering module for Trainium2 (8 NeuronCores), single-launch.

Computes, for x of shape (2, 4, 64, 256, 256) f32:
    mn/mx  = per-channel global min/max
    binary = ((x - mn) / (mx - mn)) > 0.5     [== (x - mn) > 0.5*(mx - mn)]
    dilated = 3x3x3 binary dilation of binary (SAME padding)
    out    = dilated - binary

Sharding: H (=256) split into 8 chunks of 32 rows, one per NeuronCore.
Each core receives its 32 rows plus one halo row on each side (global
edges padded with -1e30 so the halo mask is 0).  On-core layout puts
(B, D) = 128 on the SBUF partition axis; (C, H, W) live on the free axis.

Single NEFF: load shard (SWDGE q0, 16 SDMA engines), per-partition
min/max partials (DVE max, GpSimd min), PE-transpose + free-axis reduce
to [mx(4) | -mn(4)] on 8 partitions, DRAM-bounce AllReduce(max) across
the 8 cores, rank-1 PE broadcast back to 128 partitions, then per
(channel, 8-row quarter): threshold (DVE), H-dilate (DVE), W-dilate
(DVE/GpSimd), D-window count via banded PE matmul accumulating
-16*binary, saturated sigmoid (Scalar) -> exact {0,1}, SWDGE store.
"""

import os
import sys

import numpy as np

for _p in ("/opt/trn_rl_repo", "/root/.axon_site/_ro/trn_rl_repo"):
    if os.path.isdir(_p) and _p not in sys.path:
        sys.path.insert(0, _p)

import ml_dtypes

B, C, D, H, W = 2, 4, 64, 256, 256
NCORES = 8
HS = H // NCORES  # 32 own rows per core
HA = HS + 2  # rows incl halo
HPAD = np.float32(-1e30)  # halo pad at global H edges -> mask 0

_CACHE = {}


def _consts():
    bd = np.arange(128)
    b = bd // D
    d = bd % D
    A = (b[:, None] == b[None, :]) & (np.abs(d[:, None] - d[None, :]) <= 1)
    A = A.astype(ml_dtypes.bfloat16)
    negI = (-16.0 * np.eye(128)).astype(ml_dtypes.bfloat16)
    I128 = np.eye(128, dtype=np.float32)
    return A, negI, I128


def _build(variant: str = "full"):
    import concourse.bass as bass
    import concourse.bacc as bacc
    import concourse.mybir as mybir
    import concourse.tile as tile
    from contextlib import ExitStack

    f32 = mybir.dt.float32
    bf16 = mybir.dt.bfloat16
    Alu = mybir.AluOpType

    nc = bacc.Bacc(
        "TRN2",
        target_bir_lowering=False,
        debug=False,
        num_devices=NCORES,
    )

    xs = nc.dram_tensor("xs", [B, C, D, HA, W], f32, kind="ExternalInput")
    out = nc.dram_tensor("out", [B, C, D, HS, W], f32, kind="ExternalOutput")
    A_np, negI_np, I_np = _consts()
    bandA_d = nc.inline_tensor(A_np, name="bandA")
    negI_d = nc.inline_tensor(negI_np, name="negI")
    ident_d = nc.inline_tensor(I_np, name="ident")

    xsa = xs.ap()
    outa = out.ap()

    with ExitStack() as ctx:
        tc = ctx.enter_context(tile.TileContext(nc))
        pers = ctx.enter_context(tc.tile_pool(name="pers", bufs=1))
        binp = ctx.enter_context(tc.tile_pool(name="binp", bufs=2))
        mwp = ctx.enter_context(tc.tile_pool(name="mwp", bufs=2))
        sgp = ctx.enter_context(tc.tile_pool(name="sgp", bufs=2))
        psump = ctx.enter_context(tc.tile_pool(name="psum", bufs=2, space="PSUM"))
        dram = ctx.enter_context(tc.tile_pool(name="dram", bufs=1, space="DRAM"))

        x_all = pers.tile([128, C, HA, W], f32)  # 136 KiB / partition
        # H-dilated mask, double-buffered manually: rows of 258 with zero
        # pad cols 0 and 257 so the W-shift views read zeros at the edges
        mh0 = pers.tile([128, 8, 258], bf16)
        mh1 = pers.tile([128, 8, 258], bf16)
        mh = [mh0, mh1]
        pmax = pers.tile([128, 8], f32)
        pmin = pers.tile([128, 8], f32)
        red8 = pers.tile([128, 8], f32)  # [mx(4) | -mn(4)] local
        s8 = pers.tile([128, 1], f32)  # per-partition reduced (parts 0..7)
        s1v = pers.tile([128, 8], f32)  # allreduced vals on partition 0
        gv8 = pers.tile([128, 8], f32)  # broadcast [mx | -mn] on all parts
        mnv = pers.tile([128, 4], f32)  # mn per channel
        h4 = pers.tile([128, 4], f32)  # 0.5*(mx-mn) per channel
        At = pers.tile([128, 128], bf16)
        Nt = pers.tile([128, 128], bf16)
        It = pers.tile([128, 128], f32)
        ones1 = pers.tile([128, 128], f32)  # row 0 used as all-ones lhsT
        selb = pers.tile([128, 1], f32)

        ccin = dram.tile([8, 1], f32)
        ccout = dram.tile([8, 1], f32)

        nc.vector.memset(selb[:, :], -100.0)
        nc.vector.memset(ones1[:, :], 1.0)
        nc.vector.memset(mh[0][:, :, :], 0.0)
        nc.vector.memset(mh[1][:, :, :], 0.0)
        nc.sync.dma_start(out=At[:, :], in_=bandA_d.ap())
        nc.sync.dma_start(out=Nt[:, :], in_=negI_d.ap())
        nc.sync.dma_start(out=It[:, :], in_=ident_d.ap())

        # ---- phase 1: load + global min/max ----
        for k in range(8):
            c, half = k // 2, k % 2
            nc.gpsimd.dma_start(
                out=x_all[:, c, 17 * half : 17 * half + 17, :],
                in_=xsa[:, c, :, 17 * half : 17 * half + 17, :],
            )
        skip_p1 = variant == "p2"
        for k in range(8 if not skip_p1 else 0):
            c, half = k // 2, k % 2
            # rows 1..32 only: halo rows 0/33 hold -1e30 pads on edge cores
            # which must not reach the min reduction
            chunk = x_all[:, c, 1 + 16 * half : 17 + 16 * half, :]
            nc.vector.tensor_reduce(
                out=pmax[:, k : k + 1],
                in_=chunk,
                axis=mybir.AxisListType.XY,
                op=Alu.max,
            )
            nc.vector.tensor_reduce(
                out=pmin[:, k : k + 1],
                in_=chunk,
                axis=mybir.AxisListType.XY,
                op=Alu.min,
            )
        if skip_p1:
            nc.vector.memset(pmax[:, :], 5.0)
            nc.vector.memset(pmin[:, :], -5.0)
        for c in range(C):
            nc.vector.tensor_reduce(
                out=red8[:, c : c + 1],
                in_=pmax[:, 2 * c : 2 * c + 2],
                axis=mybir.AxisListType.X,
                op=Alu.max,
            )
            nc.vector.tensor_reduce(
                out=red8[:, 4 + c : 5 + c],
                in_=pmin[:, 2 * c : 2 * c + 2],
                axis=mybir.AxisListType.X,
                op=Alu.min,
            )
        # negate mins so a single max combines both in the collective
        nc.vector.tensor_scalar_mul(red8[:, 4:8], red8[:, 4:8], -1.0)
        # cross-partition max: transpose [128p, 8] -> psum [8p, 128] via PE
        pst = psump.tile([128, 2048], f32, tag="ps")
        nc.tensor.matmul(pst[0:8, 0:128], red8[:, :], It[:, :], start=True, stop=True)
        nc.vector.tensor_reduce(
            out=s8[0:8, 0:1],
            in_=pst[0:8, 0:128],
            axis=mybir.AxisListType.X,
            op=Alu.max,
        )
        # cross-core AllReduce(max) through DRAM bounce buffers
        nc.sync.dma_start(out=ccin[:, :], in_=s8[0:8, 0:1])
        if variant in ("nocc", "p2"):
            nc.gpsimd.dma_start(out=ccout[:, :], in_=ccin[:, :])
        else:
            nc.gpsimd.collective_compute(
                "AllReduce",
                Alu.max,
                replica_groups=[list(range(NCORES))],
                ins=[ccin.opt()],
                outs=[ccout.opt()],
            )
        nc.sync.dma_start(
            out=s1v[0:1, 0:8], in_=ccout[:, :].rearrange("a b -> (a b)")[None, :]
        )
        # broadcast to 128 partitions with a rank-1 matmul
        psb = psump.tile([128, 2048], f32, tag="ps")
        nc.tensor.matmul(psb[:, 0:8], ones1[0:1, :], s1v[0:1, 0:8], start=True, stop=True)
        nc.vector.tensor_copy(gv8[:, :], psb[:, 0:8])
        nc.vector.tensor_scalar_mul(mnv[:, :], gv8[:, 4:8], -1.0)
        nc.vector.tensor_add(h4[:, :], gv8[:, 0:4], gv8[:, 4:8])
        nc.vector.tensor_scalar_mul(h4[:, :], h4[:, :], 0.5)
        if variant == "dbg":
            nc.sync.dma_start(out=outa[:, 0, :, 0, 0:8], in_=red8[:, :])
            nc.sync.dma_start(out=outa[:, 0, :, 1, 0:8], in_=gv8[:, :])
            nc.sync.dma_start(out=outa[:, 0, :, 2, 0:8], in_=pmax[:, :])
            nc.sync.dma_start(out=outa[:, 0, :, 3, 0:8], in_=pmin[:, :])

        # ---- phase 2: mask, dilate, boundary per (channel, 8-row quarter) ----
        for c in range(C if variant != "dbg" else 0):
            for q in range(4):
                idx = 4 * c + q
                binq = binp.tile([128, 10, W], bf16, tag="binq")
                nc.vector.tensor_scalar(
                    out=binq[:, :, :],
                    in0=x_all[:, c, 8 * q : 8 * q + 10, :],
                    scalar1=mnv[:, c : c + 1],
                    scalar2=h4[:, c : c + 1],
                    op0=Alu.subtract,
                    op1=Alu.is_gt,
                )
                mhq = mh[idx % 2]
                mhd = mhq[:, :, 1:257]
                nc.vector.tensor_tensor(
                    out=mhd, in0=binq[:, 0:8, :], in1=binq[:, 2:10, :], op=Alu.max
                )
                nc.vector.tensor_tensor(
                    out=mhd, in0=mhd, in1=binq[:, 1:9, :], op=Alu.max
                )
                mwq = mwp.tile([128, 8, W], bf16, tag="mw")
                nc.vector.tensor_tensor(
                    out=mwq[:, :, :],
                    in0=mhq[:, :, 0:256],
                    in1=mhq[:, :, 2:258],
                    op=Alu.max,
                )
                nc.vector.tensor_tensor(
                    out=mwq[:, :, :], in0=mwq[:, :, :], in1=mhd, op=Alu.max
                )
                ps = psump.tile([128, 2048], f32, tag="ps")
                for s in range(4):
                    nc.tensor.matmul(
                        ps[:, 512 * s : 512 * s + 512],
                        At[:, :],
                        mwq[:, 2 * s : 2 * s + 2, :],
                        start=True,
                        stop=False,
                    )
                for s in range(4):
                    nc.tensor.matmul(
                        ps[:, 512 * s : 512 * s + 512],
                        Nt[:, :],
                        binq[:, 2 * s + 1 : 2 * s + 3, :],
                        start=False,
                        stop=True,
                    )
                sg = sgp.tile([128, 2048], f32, tag="sg")
                nc.scalar.activation(
                    out=sg[:, :],
                    in_=ps[:, :],
                    func=mybir.ActivationFunctionType.Sigmoid,
                    bias=selb[:, :],
                    scale=200.0,
                )
                nc.gpsimd.dma_start(
                    out=outa[:, c, :, 8 * q : 8 * q + 8, :],
                    in_=sg.rearrange("p (r w) -> p r w", w=W),
                )

    nc.compile()
    return nc


def _get_nc_single():
    if "nc1" not in _CACHE:
        _CACHE["nc1"] = _build()
    return _CACHE["nc1"]


def _make_in_maps(x: np.ndarray):
    in_maps = []
    for k in range(NCORES):
        xs = np.empty((B, C, D, HA, W), np.float32)
        lo = k * HS
        xs[:, :, :, 1 : HS + 1, :] = x[:, :, :, lo : lo + HS, :]
        if k > 0:
            xs[:, :, :, 0, :] = x[:, :, :, lo - 1, :]
        else:
            xs[:, :, :, 0, :] = HPAD
        if k < NCORES - 1:
            xs[:, :, :, HS + 1, :] = x[:, :, :, lo + HS, :]
        else:
            xs[:, :, :, HS + 1, :] = HPAD
        in_maps.append({"xs": xs})
    return in_maps


def kernel(x: np.ndarray) -> np.ndarray:
    from concourse.bass_utils import run_bass_kernel_spmd

    x = np.ascontiguousarray(np.asarray(x), dtype=np.float32)
    assert x.shape == (B, C, D, H, W)

    in_maps = _make_in_maps(x)
    res = run_bass_kernel_spmd(
        _get_nc_single(), in_maps, core_ids=list(range(NCORES))
    )
    pieces = [res.results[k]["out"] for k in range(NCORES)]
    return np.concatenate(pieces, axis=3)


if __name__ == "__main__":
    x = np.random.randn(B, C, D, H, W).astype(np.float32)
    y = kernel(x)
    print(y.shape, y.dtype, y.sum())


# revision 18
# speedup vs baseline: 1.6175x; 1.3095x over previous
"""Boundary rendering module for Trainium2 (8 NeuronCores), single-launch.

Computes, for x of shape (2, 4, 64, 256, 256) f32:
    mn/mx  = per-channel global min/max
    binary = ((x - mn) / (mx - mn)) > 0.5     [== x > (mn + mx)/2]
    dilated = 3x3x3 binary dilation of binary (SAME padding)
    out    = dilated - binary

Sharding: H (=256) split into 8 chunks of 32 rows, one per NeuronCore.
Each core receives its 32 rows plus one halo row on each side (global
edges padded with -1e30 so the halo mask is 0).  On-core layout puts
(B, D) = 128 on the SBUF partition axis; (C, H, W) live on the free axis.

Single NEFF:
  phase 1: SWDGE loads in 8-row packets (17KB packets run 5x slower than
  8KB on the SDMA read path), DVE min/max reduces interleaved per chunk,
  PE-transpose to 8 partitions, [mx(4) | -mn(4)] replicated 8x and
  exchanged with a one-hop mesh AllToAll (a ring AllReduce costs ~57us
  for 32B; AllToAll is direct), local max over cores, rank-1 PE
  broadcast.
  phase 2 per (channel, 8-row quarter): threshold on the Scalar engine
  (saturated sigmoid at scale 1e8 -> exact {0,1}), H-dilate on DVE,
  W-dilate on DVE (even quarters) or folded into the PE dw-shifted
  band matmuls (odd quarters), D-window count + -16*binary in PSUM,
  saturated sigmoid -> out, SWDGE stores.
"""

import os
import sys

import numpy as np

for _p in ("/opt/trn_rl_repo", "/root/.axon_site/_ro/trn_rl_repo"):
    if os.path.isdir(_p) and _p not in sys.path:
        sys.path.insert(0, _p)

import ml_dtypes

B, C, D, H, W = 2, 4, 64, 256, 256
NCORES = 8
HS = H // NCORES  # 32 own rows per core
HA = HS + 2  # rows incl halo
HPAD = np.float32(-1e30)  # halo pad at global H edges -> mask 0

LROWS = [(0, 9), (9, 17), (17, 25), (25, 34)]  # load chunks (<=9KB packets)
RROWS = [(1, 9), (9, 17), (17, 25), (25, 33)]  # reduce chunks (own rows only)

_CACHE = {}


def _consts():
    bd = np.arange(128)
    b = bd // D
    d = bd % D
    A = (b[:, None] == b[None, :]) & (np.abs(d[:, None] - d[None, :]) <= 1)
    A = A.astype(ml_dtypes.bfloat16)
    negI = (-16.0 * np.eye(128)).astype(ml_dtypes.bfloat16)
    I128 = np.eye(128, dtype=np.float32)
    return A, negI, I128


def _build(variant: str = "full", w_on_pe=lambda idx: idx % 2 == 1):
    import concourse.bass as bass
    import concourse.bacc as bacc
    import concourse.mybir as mybir
    import concourse.tile as tile
    from contextlib import ExitStack

    f32 = mybir.dt.float32
    bf16 = mybir.dt.bfloat16
    Alu = mybir.AluOpType
    Act = mybir.ActivationFunctionType

    nc = bacc.Bacc(
        "TRN2",
        target_bir_lowering=False,
        debug=False,
        num_devices=NCORES,
    )

    xs = nc.dram_tensor("xs", [B, C, D, HA, W], f32, kind="ExternalInput")
    out = nc.dram_tensor("out", [B, C, D, HS, W], f32, kind="ExternalOutput")
    A_np, negI_np, I_np = _consts()
    bandA_d = nc.inline_tensor(A_np, name="bandA")
    negI_d = nc.inline_tensor(negI_np, name="negI")
    ident_d = nc.inline_tensor(I_np, name="ident")

    xsa = xs.ap()
    outa = out.ap()

    with ExitStack() as ctx:
        tc = ctx.enter_context(tile.TileContext(nc))
        pers = ctx.enter_context(tc.tile_pool(name="pers", bufs=1))
        binp = ctx.enter_context(tc.tile_pool(name="binp", bufs=3))
        mwp = ctx.enter_context(tc.tile_pool(name="mwp", bufs=2))
        sgp = ctx.enter_context(tc.tile_pool(name="sgp", bufs=2))
        psump = ctx.enter_context(tc.tile_pool(name="psum", bufs=2, space="PSUM"))
        dram = ctx.enter_context(tc.tile_pool(name="dram", bufs=1, space="DRAM"))

        x_all = pers.tile([128, C, HA, W], f32)  # 136 KiB / partition
        # H-dilated mask, double-buffered manually: rows of 258 with zero
        # pad cols 0 and 257 so the W-shift views read zeros at the edges
        mh0 = pers.tile([128, 8, 258], bf16)
        mh1 = pers.tile([128, 8, 258], bf16)
        mh = [mh0, mh1]
        pmax = pers.tile([128, 16], f32)
        pmin = pers.tile([128, 16], f32)
        red8 = pers.tile([128, 8], f32)  # [mx(4) | -mn(4)] local
        s8 = pers.tile([128, 1], f32)  # per-partition reduced (parts 0..7)
        s64 = pers.tile([128, 8], f32)  # s8 replicated 8x along free axis
        z8 = pers.tile([128, 8], f32)  # zeros
        s1v = pers.tile([128, 72], f32)  # gathered (0:64) + reduced (64:72)
        gv8 = pers.tile([128, 8], f32)  # broadcast [mx | -mn] on all parts
        mnv = pers.tile([128, 4], f32)  # mn per channel
        h4 = pers.tile([128, 4], f32)  # 0.5*(mx-mn) per channel
        bias4 = pers.tile([128, 4], f32)  # -1e8 * (mn + h) per channel
        At = pers.tile([128, 128], bf16)
        Nt = pers.tile([128, 128], bf16)
        It = pers.tile([128, 128], f32)
        ones1 = pers.tile([128, 128], f32)  # row 0 used as all-ones lhsT
        selb = pers.tile([128, 1], f32)

        ccin = dram.tile([8, 8], f32)
        ccout = dram.tile([8, 8], f32)

        nc.vector.memset(selb[:, :], -100.0)
        nc.vector.memset(ones1[:, :], 1.0)
        nc.vector.memset(z8[:, :], 0.0)
        nc.vector.memset(mh0[:, :, :], 0.0)
        nc.vector.memset(mh1[:, :, :], 0.0)
        nc.sync.dma_start(out=At[:, :], in_=bandA_d.ap())
        nc.sync.dma_start(out=Nt[:, :], in_=negI_d.ap())
        nc.sync.dma_start(out=It[:, :], in_=ident_d.ap())

        # ---- phase 1: load + global min/max, interleaved per chunk ----
        skip_p1 = variant == "p2"
        for c in range(C):
            for g in range(4):
                l0, l1 = LROWS[g]
                nc.gpsimd.dma_start(
                    out=x_all[:, c, l0:l1, :],
                    in_=xsa[:, c, :, l0:l1, :],
                )
                if skip_p1:
                    continue
                r0, r1 = RROWS[g]
                k = 4 * c + g
                chunk = x_all[:, c, r0:r1, :]
                nc.vector.tensor_reduce(
                    out=pmax[:, k : k + 1],
                    in_=chunk,
                    axis=mybir.AxisListType.XY,
                    op=Alu.max,
                )
                nc.vector.tensor_reduce(
                    out=pmin[:, k : k + 1],
                    in_=chunk,
                    axis=mybir.AxisListType.XY,
                    op=Alu.min,
                )
        if skip_p1:
            nc.vector.memset(pmax[:, :], 5.0)
            nc.vector.memset(pmin[:, :], -5.0)
        for c in range(C):
            nc.vector.tensor_reduce(
                out=red8[:, c : c + 1],
                in_=pmax[:, 4 * c : 4 * c + 4],
                axis=mybir.AxisListType.X,
                op=Alu.max,
            )
            nc.vector.tensor_reduce(
                out=red8[:, 4 + c : 5 + c],
                in_=pmin[:, 4 * c : 4 * c + 4],
                axis=mybir.AxisListType.X,
                op=Alu.min,
            )
        # negate mins so a single max combines both after the exchange
        nc.vector.tensor_scalar_mul(red8[:, 4:8], red8[:, 4:8], -1.0)
        # cross-partition max: transpose [128p, 8] -> psum [8p, 128] via PE
        pst = psump.tile([128, 2048], f32, tag="ps")
        nc.tensor.matmul(pst[0:8, 0:128], red8[:, :], It[:, :], start=True, stop=True)
        nc.vector.tensor_reduce(
            out=s8[0:8, 0:1],
            in_=pst[0:8, 0:128],
            axis=mybir.AxisListType.X,
            op=Alu.max,
        )
        # replicate the 8 values 8x along the free axis (one copy per peer)
        nc.vector.tensor_scalar(
            out=s64[0:8, 0:8],
            in0=z8[0:8, 0:8],
            scalar1=s8[0:8, 0:1],
            scalar2=None,
            op0=Alu.add,
        )
        # one-hop mesh AllToAll: ccin[j, v] = myvals[v] -> ccout[k, v] =
        # core k's vals[v]; local max over k replaces a 14-hop ring
        nc.sync.dma_start(
            out=ccin[:, :].rearrange("j v -> v j"), in_=s64[0:8, 0:8]
        )
        if variant in ("nocc", "p2"):
            nc.gpsimd.dma_start(out=ccout[:, :], in_=ccin[:, :])
        else:
            nc.gpsimd.collective_compute(
                "AllToAll",
                Alu.bypass,
                replica_groups=[list(range(NCORES))],
                ins=[ccin.opt()],
                outs=[ccout.opt()],
            )
        nc.sync.dma_start(
            out=s1v[0:1, 0:64], in_=ccout[:, :].rearrange("k v -> (k v)")[None, :]
        )
        nc.vector.tensor_reduce(
            out=s1v[0:1, 64:72],
            in_=s1v[0:1, 0:64].rearrange("p (k v) -> p v k", k=NCORES),
            axis=mybir.AxisListType.X,
            op=Alu.max,
        )
        # broadcast to 128 partitions with a rank-1 matmul
        psb = psump.tile([128, 2048], f32, tag="ps")
        nc.tensor.matmul(
            psb[:, 0:8], ones1[0:1, :], s1v[0:1, 64:72], start=True, stop=True
        )
        nc.vector.tensor_copy(gv8[:, :], psb[:, 0:8])
        nc.vector.tensor_scalar_mul(mnv[:, :], gv8[:, 4:8], -1.0)
        nc.vector.tensor_add(h4[:, :], gv8[:, 0:4], gv8[:, 4:8])
        nc.vector.tensor_scalar_mul(h4[:, :], h4[:, :], 0.5)
        # threshold bias for the scalar engine: sigmoid(1e8*(x - (mn+h)))
        nc.vector.tensor_add(bias4[:, :], mnv[:, :], h4[:, :])
        nc.vector.tensor_scalar_mul(bias4[:, :], bias4[:, :], -1.0e8)
        if variant == "dbg":
            nc.sync.dma_start(out=outa[:, 0, :, 0, 0:8], in_=red8[:, :])
            nc.sync.dma_start(out=outa[:, 0, :, 1, 0:8], in_=gv8[:, :])
            nc.sync.dma_start(out=outa[:, 0, :, 2, 0:8], in_=pmax[:, 0:8])
            nc.sync.dma_start(out=outa[:, 0, :, 3, 0:8], in_=pmin[:, 0:8])

        # ---- phase 2: mask, dilate, boundary per (channel, 8-row quarter) ----
        def emit_thresh(idx):
            c, q = idx // 4, idx % 4
            binq = binp.tile([128, 10, W], bf16, tag="binq")
            nc.scalar.activation(
                out=binq[:, :, :],
                in_=x_all[:, c, 8 * q : 8 * q + 10, :],
                func=Act.Sigmoid,
                bias=bias4[:, c : c + 1],
                scale=1.0e8,
            )
            return binq

        def emit_rest(idx, binq):
            c, q = idx // 4, idx % 4
            mhq = mh[idx % 2]
            mhd = mhq[:, :, 1:257]
            nc.vector.tensor_tensor(
                out=mhd, in0=binq[:, 0:8, :], in1=binq[:, 2:10, :], op=Alu.max
            )
            nc.vector.tensor_tensor(
                out=mhd, in0=mhd, in1=binq[:, 1:9, :], op=Alu.max
            )
            ps = psump.tile([128, 2048], f32, tag="ps")
            if w_on_pe(idx):
                # W-dilation folded into PE: 3 dw-shifted band matmuls
                for s in range(4):
                    for j, dw in enumerate((0, 1, 2)):
                        nc.tensor.matmul(
                            ps[:, 512 * s : 512 * s + 512],
                            At[:, :],
                            mhq[:, 2 * s : 2 * s + 2, dw : dw + 256],
                            start=(j == 0),
                            stop=False,
                        )
            else:
                mwq = mwp.tile([128, 8, W], bf16, tag="mw")
                nc.vector.tensor_tensor(
                    out=mwq[:, :, :],
                    in0=mhq[:, :, 0:256],
                    in1=mhq[:, :, 2:258],
                    op=Alu.max,
                )
                nc.vector.tensor_tensor(
                    out=mwq[:, :, :], in0=mwq[:, :, :], in1=mhd, op=Alu.max
                )
                for s in range(4):
                    nc.tensor.matmul(
                        ps[:, 512 * s : 512 * s + 512],
                        At[:, :],
                        mwq[:, 2 * s : 2 * s + 2, :],
                        start=True,
                        stop=False,
                    )
            for s in range(4):
                nc.tensor.matmul(
                    ps[:, 512 * s : 512 * s + 512],
                    Nt[:, :],
                    binq[:, 2 * s + 1 : 2 * s + 3, :],
                    start=False,
                    stop=True,
                )
            sg = sgp.tile([128, 2048], f32, tag="sg")
            nc.scalar.activation(
                out=sg[:, :],
                in_=ps[:, :],
                func=Act.Sigmoid,
                bias=selb[:, :],
                scale=200.0,
            )
            nc.gpsimd.dma_start(
                out=outa[:, c, :, 8 * q : 8 * q + 8, :],
                in_=sg.rearrange("p (r w) -> p r w", w=W),
            )

        nquart = 16 if variant != "dbg" else 0
        prev = None
        for idx in range(nquart):
            binq = emit_thresh(idx)
            if prev is not None:
                emit_rest(*prev)
            prev = (idx, binq)
        if prev is not None:
            emit_rest(*prev)

    nc.compile()
    return nc


def _get_nc_single():
    if "nc1" not in _CACHE:
        _CACHE["nc1"] = _build()
    return _CACHE["nc1"]


def _make_in_maps(x: np.ndarray):
    in_maps = []
    for k in range(NCORES):
        xs = np.empty((B, C, D, HA, W), np.float32)
        lo = k * HS
        xs[:, :, :, 1 : HS + 1, :] = x[:, :, :, lo : lo + HS, :]
        if k > 0:
            xs[:, :, :, 0, :] = x[:, :, :, lo - 1, :]
        else:
            xs[:, :, :, 0, :] = HPAD
        if k < NCORES - 1:
            xs[:, :, :, HS + 1, :] = x[:, :, :, lo + HS, :]
        else:
            xs[:, :, :, HS + 1, :] = HPAD
        in_maps.append({"xs": xs})
    return in_maps


def kernel(x: np.ndarray) -> np.ndarray:
    from concourse.bass_utils import run_bass_kernel_spmd

    x = np.ascontiguousarray(np.asarray(x), dtype=np.float32)
    assert x.shape == (B, C, D, H, W)

    in_maps = _make_in_maps(x)
    res = run_bass_kernel_spmd(
        _get_nc_single(), in_maps, core_ids=list(range(NCORES))
    )
    pieces = [res.results[k]["out"] for k in range(NCORES)]
    return np.concatenate(pieces, axis=3)


if __name__ == "__main__":
    x = np.random.randn(B, C, D, H, W).astype(np.float32)
    y = kernel(x)
    print(y.shape, y.dtype, y.sum())
